# revision 21
# baseline (speedup 1.0000x reference)
"""TRN2 Bass kernel: batch-invariant full attention.

Problem: out = softmax(Q K^T / sqrt(64)) V with Q,K,V f32 [4, 16, 2048, 64].
Sharding: the 64 (batch, head) pairs are split 8 ways across the 8
NeuronCores (8 pairs per core); attention is independent per pair.

Per-core design (per pair):
  - Sequence indices are permuted as s = p*T + t so every DMA is
    contiguous per partition; the permutation is consistent between K and
    V (softmax invariant) and undone on the output write.
  - All matmul operands are fp16 (11-bit mantissa ~ tf32-grade rounding,
    2-byte so the PE streams 1 row/cycle; fp32 accumulation in PSUM).
  - Q^T / K^T are built with fp16 PE pair-transposes [128,128] -> fp16
    PSUM, giving a row-pair interleaved layout: even tiles on partitions
    0-63, odd on 64-127. QK matmuls run as concurrent row-group pairs;
    a partition-swapped DMA copy (kt2s) covers the parity cross terms.
  - Scores come out transposed, S^T[k, q]; exp on ScalarE (PSUM->SBUF,
    scale folded into the activation affine, fp16 out).
  - PV uses the exp tile as the *stationary* operand (fp16 128-col
    weights -> fast weight load) and V (augmented with a ones column)
    as the 65-row moving operand, accumulating [O ; denom] directly in
    q-major layout [128, 65] per q block. No output transposes.
  - Epilogue: reciprocal of the denom column + tensor_scalar multiply
    straight out of PSUM, contiguous DMA out.
"""
import functools
from contextlib import ExitStack

import numpy as np

import concourse.mybir as mybir
import concourse.tile as tile
from concourse import bacc
from concourse.bass_utils import run_bass_kernel_spmd
from concourse.masks import make_identity

F32 = mybir.dt.float32
F16 = mybir.dt.float16
EXP = mybir.ActivationFunctionType.Exp

B, H, S, D = 4, 16, 2048, 64
N_CORES = 8
NBH = B * H // N_CORES  # 8 (b,h) pairs per core


def build_attention(nbh=NBH, S=S, D=D):
    assert D == 64
    T = S // 128  # 16 k/q tiles of 128
    M = T // 2  # 8 tile pairs
    QCN = 2  # q chunks (1024 each for S=2048)
    qhalf = S // QCN // 2  # 512: even-parity half of a q chunk
    assert qhalf % 512 == 0  # row-pair outputs must land in distinct PSUM banks
    nblk = qhalf // 128  # 4 q blocks per parity half
    scale = 1.0 / float(np.sqrt(D))

    nc = bacc.Bacc("TRN2", target_bir_lowering=False, debug=False)
    q = nc.dram_tensor("q", [nbh, S, D], F32, kind="ExternalInput").ap()
    k = nc.dram_tensor("k", [nbh, S, D], F32, kind="ExternalInput").ap()
    v = nc.dram_tensor("v", [nbh, S, D], F32, kind="ExternalInput").ap()
    o = nc.dram_tensor("o", [nbh, S, D], F32, kind="ExternalOutput").ap()

    with tile.TileContext(nc) as tc, ExitStack() as ctx:
        singles = ctx.enter_context(tc.tile_pool(name="singles", bufs=1))
        ident = singles.tile([128, 128], F16)
        make_identity(nc, ident)

        ld = ctx.enter_context(tc.tile_pool(name="ld", bufs=2))
        c16 = ctx.enter_context(tc.tile_pool(name="c16", bufs=2))
        persist = ctx.enter_context(tc.tile_pool(name="persist", bufs=2))
        epool = ctx.enter_context(tc.tile_pool(name="epool", bufs=4))
        opool = ctx.enter_context(tc.tile_pool(name="opool", bufs=2))
        pp_s = ctx.enter_context(tc.tile_pool(name="pp_s", bufs=2, space="PSUM"))
        pp_t = ctx.enter_context(tc.tile_pool(name="pp_t", bufs=2, space="PSUM"))
        pp_o = ctx.enter_context(tc.tile_pool(name="pp_o", bufs=1, space="PSUM"))

        for bh in range(nbh):
            # ---- load (f32, contiguous via the s = p*T + t permutation) ----
            qn = ld.tile([128, T, D], F32, tag="qn")
            kn = ld.tile([128, T, D], F32, tag="kn")
            vraw = ld.tile([128, T, D + 1], F32, tag="vraw")
            qv = q[bh].rearrange("(p t) d -> p t d", p=128)
            kv = k[bh].rearrange("(p t) d -> p t d", p=128)
            if bh == 0:
                H2 = T // 4
                nc.gpsimd.dma_start(out=qn[:, 0:H2, :], in_=qv[:, 0:H2, :])
                nc.gpsimd.dma_start(out=kn[:, 0:H2, :], in_=kv[:, 0:H2, :])
                nc.gpsimd.dma_start(out=qn[:, H2:T, :], in_=qv[:, H2:T, :])
                nc.gpsimd.dma_start(out=kn[:, H2:T, :], in_=kv[:, H2:T, :])
            else:
                nc.gpsimd.dma_start(out=qn, in_=qv)
                nc.gpsimd.dma_start(out=kn, in_=kv)
            nc.gpsimd.dma_start(
                out=vraw[:, :, 0:D], in_=v[bh].rearrange("(p t) d -> p t d", p=128)
            )
            nc.gpsimd.memset(vraw[:, :, D : D + 1], 1.0)

            # ---- fp16 casts (DVE) ----
            qn16 = c16.tile([128, T, D], F16, tag="qn16")
            kn16 = c16.tile([128, T, D], F16, tag="kn16")
            vaug = persist.tile([128, T, D + 1], F16, tag="vaug")
            # scale*log2(e) folded into the Q cast: scores become log2-scaled,
            # so exp is computed as 2^z = e^(z*ln2) (DVE fast-exp ready).
            sc = float(scale * np.log2(np.e))
            if bh == 0:
                nc.vector.tensor_scalar_mul(out=qn16[:, 0:H2, :], in0=qn[:, 0:H2, :], scalar1=sc)
                nc.vector.tensor_copy(out=kn16[:, 0:H2, :], in_=kn[:, 0:H2, :])
                nc.vector.tensor_scalar_mul(out=qn16[:, H2:T, :], in0=qn[:, H2:T, :], scalar1=sc)
                nc.vector.tensor_copy(out=kn16[:, H2:T, :], in_=kn[:, H2:T, :])
            else:
                nc.vector.tensor_scalar_mul(out=qn16, in0=qn, scalar1=sc)
                nc.vector.tensor_copy(out=kn16, in_=kn)
            nc.vector.tensor_copy(out=vaug, in_=vraw)

            # ---- PE pair-transposes: qt2/kt2 [128, M, 128] interleaved ----
            # qt2[0:64, m, j] = Q^T[d, q tile 2m, col j] (tile col j <-> s = j*T + 2m)
            # qt2[64:128, m, j] = Q^T[d, q tile 2m+1, col j]
            qt2 = persist.tile([128, M, 128], F16, tag="qt2")
            kt2 = persist.tile([128, M, 128], F16, tag="kt2")
            kt2s = persist.tile([128, M, 128], F16, tag="kt2s")
            for m in range(M):
                # two transposes share one 1-bank psum tile (4 slots total)
                ptb = pp_t.tile([128, 2, 128], F16, tag="ptr", name=f"ptb{bh}_{m}")
                nc.tensor.transpose(
                    out=ptb[:, 0, :], in_=qn16[:, 2 * m : 2 * m + 2, :], identity=ident
                )
                nc.vector.tensor_copy(out=qt2[:, m, :], in_=ptb[:, 0, :])
                nc.tensor.transpose(
                    out=ptb[:, 1, :], in_=kn16[:, 2 * m : 2 * m + 2, :], identity=ident
                )
                nc.vector.tensor_copy(out=kt2[:, m, :], in_=ptb[:, 1, :])
                # per-pair partition-swapped copy for the parity cross terms,
                # so QK iteration m only depends on its own tiles
                nc.gpsimd.dma_start(out=kt2s[0:64, m, :], in_=kt2[64:128, m, :])
                nc.gpsimd.dma_start(out=kt2s[64:128, m, :], in_=kt2[0:64, m, :])

            qt2f = qt2.rearrange("p m j -> p (m j)")
            kt2f = kt2.rearrange("p m j -> p (m j)")
            kt2sf = kt2s.rearrange("p m j -> p (m j)")

            # ---- QK -> exp -> PV ----
            for qc in range(QCN):
                # poq[:, c, :] accumulates [O ; denom] for q block c of this
                # chunk: c < nblk are even-parity q tiles, c >= nblk odd.
                # padded to 128 f32 per block so each [128, 65] matmul output
                # stays inside one 2KB PSUM bank
                poq = pp_o.tile([128, 2 * nblk, 128], F32, tag="poq")
                rhs_lo = qt2f[0:64, qc * qhalf : (qc + 1) * qhalf]
                rhs_hi = qt2f[64:128, qc * qhalf : (qc + 1) * qhalf]
                for m in range(M):
                    for cross in (0, 1):
                        kk = kt2sf if cross else kt2f
                        kb_lo = 2 * m + cross
                        kb_hi = 2 * m + 1 - cross
                        ps = pp_s.tile([128, 2 * qhalf], F32, tag="ps")
                        nc.tensor.matmul(
                            out=ps[:, 0:qhalf],
                            lhsT=kk[0:64, 128 * m : 128 * (m + 1)],
                            rhs=rhs_lo,
                            start=True,
                            stop=True,
                        )
                        nc.tensor.matmul(
                            out=ps[:, qhalf : 2 * qhalf],
                            lhsT=kk[64:128, 128 * m : 128 * (m + 1)],
                            rhs=rhs_hi,
                            start=True,
                            stop=True,
                        )
                        e = epool.tile([128, 2 * qhalf], F16, tag="e")
                        nc.scalar.activation(out=e, in_=ps, func=EXP, scale=float(np.log(2.0)))
                        first = m == 0 and cross == 0
                        last = m == M - 1 and cross == 1
                        for c in range(2 * nblk):
                            kb = kb_lo if c < nblk else kb_hi
                            # start=True clears the ENTIRE psum bank, so only
                            # the first matmul touching each bank may set it;
                            # per-element has_written handles the other blocks.
                            nc.tensor.matmul(
                                out=poq[:, c, 0 : D + 1],
                                lhsT=e[:, 128 * c : 128 * (c + 1)],
                                rhs=vaug[:, kb, :],
                                start=first and c % nblk == 0,
                                stop=last,
                            )

                # ---- epilogue: one copy frees poq's PSUM banks early, then
                # normalize from SBUF ----
                ocp = opool.tile([128, 2 * nblk, D + 1], F32, tag="ocp")
                nc.vector.tensor_copy(out=ocp, in_=poq[:, :, 0 : D + 1])
                outsb = opool.tile([128, 2 * nblk, D], F32, tag="outsb")
                for c in range(2 * nblk):
                    tt_local = 2 * c if c < nblk else 2 * (c - nblk) + 1
                    rcp = opool.tile([128, 1], F32, tag="rcp")
                    nc.vector.reciprocal(out=rcp, in_=ocp[:, c, D : D + 1])
                    nc.vector.tensor_scalar_mul(
                        out=outsb[:, tt_local, :], in0=ocp[:, c, 0:D], scalar1=rcp
                    )
                nc.gpsimd.dma_start(
                    out=o[bh].rearrange("(p t) d -> p t d", p=128)[
                        :, qc * 2 * nblk : (qc + 1) * 2 * nblk, :
                    ],
                    in_=outsb,
                )
    nc.compile()
    return nc


@functools.lru_cache(maxsize=1)
def _built():
    return build_attention()


def run(query, key, value, trace=False):
    """Shard (b,h) pairs 8 ways, run on cores 0-7, gather. Returns
    (out [B,H,S,D] f32, BassKernelResults)."""
    nc = _built()
    qf = np.ascontiguousarray(np.asarray(query, dtype=np.float32).reshape(B * H, S, D))
    kf = np.ascontiguousarray(np.asarray(key, dtype=np.float32).reshape(B * H, S, D))
    vf = np.ascontiguousarray(np.asarray(value, dtype=np.float32).reshape(B * H, S, D))
    in_maps = []
    for c in range(N_CORES):
        sl = slice(c * NBH, (c + 1) * NBH)
        in_maps.append(
            {
                "q": np.ascontiguousarray(qf[sl]),
                "k": np.ascontiguousarray(kf[sl]),
                "v": np.ascontiguousarray(vf[sl]),
            }
        )
    res = None
    last_err = None
    for attempt in range(3):
        try:
            res = run_bass_kernel_spmd(
                nc, in_maps, core_ids=list(range(N_CORES)), trace=trace
            )
            break
        except Exception as e:  # transient device wedge: retry
            last_err = e
            import time as _time

            _time.sleep(5 * (attempt + 1))
    if res is None:
        raise last_err
    out = np.concatenate([res.results[c]["o"] for c in range(N_CORES)], axis=0)
    return out.reshape(B, H, S, D).astype(np.float32), res


def kernel(query, key, value):
    out, _ = run(query, key, value)
    return out


# revision 22
# speedup vs baseline: 1.0079x; 1.0079x over previous
"""TRN2 Bass kernel: batch-invariant full attention.

Problem: out = softmax(Q K^T / sqrt(64)) V with Q,K,V f32 [4, 16, 2048, 64].
Sharding: the 64 (batch, head) pairs are split 8 ways across the 8
NeuronCores (8 pairs per core); attention is independent per pair.

Per-core design (per pair):
  - Sequence indices are permuted as s = p*T + t so every DMA is
    contiguous per partition; the permutation is consistent between K and
    V (softmax invariant) and undone on the output write.
  - All matmul operands are fp16 (11-bit mantissa ~ tf32-grade rounding,
    2-byte so the PE streams 1 row/cycle; fp32 accumulation in PSUM).
  - Q^T / K^T are built with fp16 PE pair-transposes [128,128] -> fp16
    PSUM, giving a row-pair interleaved layout: even tiles on partitions
    0-63, odd on 64-127. QK matmuls run as concurrent row-group pairs;
    a partition-swapped DMA copy (kt2s) covers the parity cross terms.
  - Scores come out transposed, S^T[k, q]; exp on ScalarE (PSUM->SBUF,
    scale folded into the activation affine, fp16 out).
  - PV uses the exp tile as the *stationary* operand (fp16 128-col
    weights -> fast weight load) and V (augmented with a ones column)
    as the 65-row moving operand, accumulating [O ; denom] directly in
    q-major layout [128, 65] per q block. No output transposes.
  - Epilogue: reciprocal of the denom column + tensor_scalar multiply
    straight out of PSUM, contiguous DMA out.
"""
import functools
from contextlib import ExitStack

import numpy as np

import concourse.mybir as mybir
import concourse.tile as tile
from concourse import bacc
from concourse.bass_utils import run_bass_kernel_spmd
from concourse.masks import make_identity

F32 = mybir.dt.float32
F16 = mybir.dt.float16
EXP = mybir.ActivationFunctionType.Exp

B, H, S, D = 4, 16, 2048, 64
N_CORES = 8
NBH = B * H // N_CORES  # 8 (b,h) pairs per core


def build_attention(nbh=NBH, S=S, D=D):
    assert D == 64
    T = S // 128  # 16 k/q tiles of 128
    M = T // 2  # 8 tile pairs
    QCN = 2  # q chunks (1024 each for S=2048)
    qhalf = S // QCN // 2  # 512: even-parity half of a q chunk
    assert qhalf % 512 == 0  # row-pair outputs must land in distinct PSUM banks
    nblk = qhalf // 128  # 4 q blocks per parity half
    scale = 1.0 / float(np.sqrt(D))

    nc = bacc.Bacc("TRN2", target_bir_lowering=False, debug=False)
    q = nc.dram_tensor("q", [nbh, S, D], F32, kind="ExternalInput").ap()
    k = nc.dram_tensor("k", [nbh, S, D], F32, kind="ExternalInput").ap()
    v = nc.dram_tensor("v", [nbh, S, D], F32, kind="ExternalInput").ap()
    o = nc.dram_tensor("o", [nbh, S, D], F32, kind="ExternalOutput").ap()

    with tile.TileContext(nc) as tc, ExitStack() as ctx:
        singles = ctx.enter_context(tc.tile_pool(name="singles", bufs=1))
        ident = singles.tile([128, 128], F16)
        make_identity(nc, ident)

        ld = ctx.enter_context(tc.tile_pool(name="ld", bufs=2))
        c16 = ctx.enter_context(tc.tile_pool(name="c16", bufs=2))
        persist = ctx.enter_context(tc.tile_pool(name="persist", bufs=2))
        epool = ctx.enter_context(tc.tile_pool(name="epool", bufs=4))
        opool = ctx.enter_context(tc.tile_pool(name="opool", bufs=2))
        pp_s = ctx.enter_context(tc.tile_pool(name="pp_s", bufs=2, space="PSUM"))
        pp_t = ctx.enter_context(tc.tile_pool(name="pp_t", bufs=2, space="PSUM"))
        pp_o = ctx.enter_context(tc.tile_pool(name="pp_o", bufs=1, space="PSUM"))

        for bh in range(nbh):
            # ---- load (f32, contiguous via the s = p*T + t permutation) ----
            qn = ld.tile([128, T, D], F32, tag="qn")
            kn = ld.tile([128, T, D], F32, tag="kn")
            vraw = ld.tile([128, T, D + 1], F32, tag="vraw")
            qv = q[bh].rearrange("(p t) d -> p t d", p=128)
            kv = k[bh].rearrange("(p t) d -> p t d", p=128)
            if bh == 0:
                H2 = T // 4
                nc.gpsimd.dma_start(out=qn[:, 0:H2, :], in_=qv[:, 0:H2, :])
                nc.gpsimd.dma_start(out=kn[:, 0:H2, :], in_=kv[:, 0:H2, :])
                nc.gpsimd.dma_start(out=qn[:, H2:T, :], in_=qv[:, H2:T, :])
                nc.gpsimd.dma_start(out=kn[:, H2:T, :], in_=kv[:, H2:T, :])
            else:
                nc.gpsimd.dma_start(out=qn, in_=qv)
                nc.gpsimd.dma_start(out=kn, in_=kv)
            nc.gpsimd.dma_start(
                out=vraw[:, :, 0:D], in_=v[bh].rearrange("(p t) d -> p t d", p=128)
            )
            nc.gpsimd.memset(vraw[:, :, D : D + 1], 1.0)

            # ---- fp16 casts (DVE) ----
            qn16 = c16.tile([128, T, D], F16, tag="qn16")
            kn16 = c16.tile([128, T, D], F16, tag="kn16")
            vaug = persist.tile([128, T, D + 1], F16, tag="vaug")
            # scale*log2(e) folded into the Q cast: scores become log2-scaled,
            # so exp is computed as 2^z = e^(z*ln2) (DVE fast-exp ready).
            sc = float(scale * np.log2(np.e))
            if bh == 0:
                nc.vector.tensor_scalar_mul(out=qn16[:, 0:H2, :], in0=qn[:, 0:H2, :], scalar1=sc)
                nc.vector.tensor_copy(out=kn16[:, 0:H2, :], in_=kn[:, 0:H2, :])
                nc.vector.tensor_scalar_mul(out=qn16[:, H2:T, :], in0=qn[:, H2:T, :], scalar1=sc)
                nc.vector.tensor_copy(out=kn16[:, H2:T, :], in_=kn[:, H2:T, :])
            else:
                nc.vector.tensor_scalar_mul(out=qn16, in0=qn, scalar1=sc)
                nc.vector.tensor_copy(out=kn16, in_=kn)
            nc.vector.tensor_copy(out=vaug, in_=vraw)

            # ---- PE pair-transposes: qt2/kt2 [128, M, 128] interleaved ----
            # qt2[0:64, m, j] = Q^T[d, q tile 2m, col j] (tile col j <-> s = j*T + 2m)
            # qt2[64:128, m, j] = Q^T[d, q tile 2m+1, col j]
            qt2 = persist.tile([128, M, 128], F16, tag="qt2")
            kt2 = persist.tile([128, M, 128], F16, tag="kt2")
            kt2s = persist.tile([128, M, 128], F16, tag="kt2s")
            for m in range(M):
                ptq = pp_t.tile([128, 128], F16, tag="ptr", name=f"ptq{bh}_{m}")
                nc.tensor.transpose(
                    out=ptq, in_=qn16[:, 2 * m : 2 * m + 2, :], identity=ident
                )
                nc.vector.tensor_copy(out=qt2[:, m, :], in_=ptq)
                ptk = pp_t.tile([128, 128], F16, tag="ptr", name=f"ptk{bh}_{m}")
                nc.tensor.transpose(
                    out=ptk, in_=kn16[:, 2 * m : 2 * m + 2, :], identity=ident
                )
                nc.vector.tensor_copy(out=kt2[:, m, :], in_=ptk)
                # per-pair partition-swapped copy for the parity cross terms,
                # so QK iteration m only depends on its own tiles
                nc.gpsimd.dma_start(out=kt2s[0:64, m, :], in_=kt2[64:128, m, :])
                nc.gpsimd.dma_start(out=kt2s[64:128, m, :], in_=kt2[0:64, m, :])

            qt2f = qt2.rearrange("p m j -> p (m j)")
            kt2f = kt2.rearrange("p m j -> p (m j)")
            kt2sf = kt2s.rearrange("p m j -> p (m j)")

            # ---- QK -> exp -> PV ----
            for qc in range(QCN):
                # poq[:, c, :] accumulates [O ; denom] for q block c of this
                # chunk: c < nblk are even-parity q tiles, c >= nblk odd.
                # padded to 128 f32 per block so each [128, 65] matmul output
                # stays inside one 2KB PSUM bank
                poq = pp_o.tile([128, 2 * nblk, 128], F32, tag="poq")
                rhs_lo = qt2f[0:64, qc * qhalf : (qc + 1) * qhalf]
                rhs_hi = qt2f[64:128, qc * qhalf : (qc + 1) * qhalf]
                for m in range(M):
                    for cross in (0, 1):
                        kk = kt2sf if cross else kt2f
                        kb_lo = 2 * m + cross
                        kb_hi = 2 * m + 1 - cross
                        ps = pp_s.tile([128, 2 * qhalf], F32, tag="ps")
                        nc.tensor.matmul(
                            out=ps[:, 0:qhalf],
                            lhsT=kk[0:64, 128 * m : 128 * (m + 1)],
                            rhs=rhs_lo,
                            start=True,
                            stop=True,
                        )
                        nc.tensor.matmul(
                            out=ps[:, qhalf : 2 * qhalf],
                            lhsT=kk[64:128, 128 * m : 128 * (m + 1)],
                            rhs=rhs_hi,
                            start=True,
                            stop=True,
                        )
                        e = epool.tile([128, 2 * qhalf], F16, tag="e")
                        nc.scalar.activation(out=e, in_=ps, func=EXP, scale=float(np.log(2.0)))
                        first = m == 0 and cross == 0
                        last = m == M - 1 and cross == 1
                        for c in range(2 * nblk):
                            kb = kb_lo if c < nblk else kb_hi
                            # start=True clears the ENTIRE psum bank, so only
                            # the first matmul touching each bank may set it;
                            # per-element has_written handles the other blocks.
                            nc.tensor.matmul(
                                out=poq[:, c, 0 : D + 1],
                                lhsT=e[:, 128 * c : 128 * (c + 1)],
                                rhs=vaug[:, kb, :],
                                start=first and c % nblk == 0,
                                stop=last,
                            )

                # ---- epilogue: one copy frees poq's PSUM banks early, then
                # normalize from SBUF ----
                ocp = opool.tile([128, 2 * nblk, D + 1], F32, tag="ocp")
                nc.vector.tensor_copy(out=ocp, in_=poq[:, :, 0 : D + 1])
                outsb = opool.tile([128, 2 * nblk, D], F32, tag="outsb")
                for c in range(2 * nblk):
                    tt_local = 2 * c if c < nblk else 2 * (c - nblk) + 1
                    rcp = opool.tile([128, 1], F32, tag="rcp")
                    nc.vector.reciprocal(out=rcp, in_=ocp[:, c, D : D + 1])
                    nc.vector.tensor_scalar_mul(
                        out=outsb[:, tt_local, :], in0=ocp[:, c, 0:D], scalar1=rcp
                    )
                nc.gpsimd.dma_start(
                    out=o[bh].rearrange("(p t) d -> p t d", p=128)[
                        :, qc * 2 * nblk : (qc + 1) * 2 * nblk, :
                    ],
                    in_=outsb,
                )
    nc.compile()
    return nc


@functools.lru_cache(maxsize=1)
def _built():
    return build_attention()


def run(query, key, value, trace=False):
    """Shard (b,h) pairs 8 ways, run on cores 0-7, gather. Returns
    (out [B,H,S,D] f32, BassKernelResults)."""
    nc = _built()
    qf = np.ascontiguousarray(np.asarray(query, dtype=np.float32).reshape(B * H, S, D))
    kf = np.ascontiguousarray(np.asarray(key, dtype=np.float32).reshape(B * H, S, D))
    vf = np.ascontiguousarray(np.asarray(value, dtype=np.float32).reshape(B * H, S, D))
    in_maps = []
    for c in range(N_CORES):
        sl = slice(c * NBH, (c + 1) * NBH)
        in_maps.append(
            {
                "q": np.ascontiguousarray(qf[sl]),
                "k": np.ascontiguousarray(kf[sl]),
                "v": np.ascontiguousarray(vf[sl]),
            }
        )
    res = None
    last_err = None
    for attempt in range(3):
        try:
            res = run_bass_kernel_spmd(
                nc, in_maps, core_ids=list(range(N_CORES)), trace=trace
            )
            break
        except Exception as e:  # transient device wedge: retry
            last_err = e
            import time as _time

            _time.sleep(5 * (attempt + 1))
    if res is None:
        raise last_err
    out = np.concatenate([res.results[c]["o"] for c in range(N_CORES)], axis=0)
    return out.reshape(B, H, S, D).astype(np.float32), res


def kernel(query, key, value):
    out, _ = run(query, key, value)
    return out
